# revision 20
# baseline (speedup 1.0000x reference)
"""Trainium2 Bass kernel for nn_EquivariantMLP (GNN message passing).

Strategy (8 NeuronCores, edge-parallel by destination node):
- Host: sort edges by destination (row), partition nodes into 8 contiguous
  ranges with their incoming edges; pack each core's edges into groups of
  1024 whose rows span a <128-node window (nodes never split across groups);
  distribute per-core edge data (edge_attr, edge_vec) in sorted order.
- Device phase 0: cast/transform s into an f16 table AB = [s@W1a | s@W1b].
- Device phase 1 (per 1024-edge group): dma_gather AB[row]/AB[col], add,
  transpose to feature-major, += edge_attr@W1c, silu -> u; msg = u@W2 in both
  feature-major (for silu(msg+b2)@Wv path) and edge-major (for scatter);
  scatter via onehot matmul into PSUM window accumulator; indirect-scatter
  the window rows to a DRAM accumulator (disjoint windows).
- Device phase 2: node update: s' = LN(s + silu(s_out)@Ws + bs), v' = v+v_out.
  Biases b2/bv enter via deg*b2 and bv (x) segsum(ev) corrections (linearity).
"""

import sys

sys.path.insert(0, "/opt/trn_rl_repo")

import numpy as np

import concourse.bass as bass
import concourse.bacc as bacc
import concourse.tile as tile
from concourse import mybir
from concourse.bass_utils import run_bass_kernel_spmd

f16 = mybir.dt.float16
f32 = mybir.dt.float32
i16 = mybir.dt.int16
i32 = mybir.dt.int32

N, E, DS, DV = 20000, 640000, 128, 64
NCORES = 8
P = 128
GE = 1024          # edges per group (8 tiles of 128)
NT = GE // P       # tiles per group = 8
NPC = N // NCORES  # nodes per core
NPC_PAD = ((NPC + P - 1) // P) * P          # 2560
NTILE_NODE = NPC_PAD // P                   # 20
N_PAD = ((N + P - 1) // P) * P              # 20096
EPS = 1e-5
BIG = 1 << 28
ACCW = 323  # 128 msg + 192 vmsg + 3 ev


def _wrap_idx(idx_lin):
    """Wrap linear int16 indices for dma_gather: [i%16, i//16], tiled to 128."""
    n = len(idx_lin)
    a = np.asarray(idx_lin, np.int16).reshape(n // 16, 16).T  # [16, n//16]
    return np.tile(a, (8, 1))


def _host_prep(s, v, edge_index, edge_attr, edge_vec_unit):
    row = np.asarray(edge_index[0], np.int64)
    col = np.asarray(edge_index[1], np.int64)
    perm = np.argsort(row, kind="stable")
    row_s = row[perm]
    col_s = col[perm]

    core_edge_bounds = np.searchsorted(row_s, np.arange(NCORES + 1) * NPC)
    deg_all = np.bincount(row, minlength=N).astype(np.int64)
    assert (deg_all > 0).all(), "degree-0 node present; grouping patch needed"

    # ---- group edges per core ----
    cores = []
    for k in range(NCORES):
        lo, hi = core_edge_bounds[k], core_edge_bounds[k + 1]
        nodes = np.arange(k * NPC, (k + 1) * NPC)
        counts = deg_all[nodes]
        groups = []  # (base_node, estart, eend) edge offsets relative to lo
        cur_base, cur_start, cur_len = None, 0, 0
        pos = 0
        for n_, c_ in zip(nodes, counts):
            c_ = int(c_)
            if cur_base is None or cur_len + c_ > GE or (n_ - cur_base) >= P:
                if cur_base is not None:
                    groups.append((cur_base, cur_start, pos))
                cur_base, cur_start, cur_len = int(n_), pos, 0
            cur_len += c_
            pos += c_
        groups.append((cur_base, cur_start, pos))
        assert pos == hi - lo
        cores.append((lo, hi, groups))

    G = max(len(c[2]) for c in cores)

    in_maps = []
    for k in range(NCORES):
        lo, hi, groups = cores[k]
        core_base = k * NPC
        idx16 = np.zeros((G, P, 2 * GE // 16), np.int16)
        meta32 = np.zeros((G, P, NT, 4), np.float32)
        meta32[:, :, :, 0] = 300.0
        ev16 = np.zeros((G, P, NT, 3), np.float16)
        ea_s = np.zeros((G * GE, DS), np.float32)
        scat32 = np.full((G, P), BIG, np.int32)
        for g, (base, es, ee) in enumerate(groups):
            ne = ee - es
            eids = perm[lo + es : lo + ee]
            r = row_s[lo + es : lo + ee]
            c = col_s[lo + es : lo + ee]
            rpad = np.zeros(GE, np.int64)
            cpad = np.zeros(GE, np.int64)
            rpad[:ne] = r
            cpad[:ne] = c
            idx16[g, :, : GE // 16] = _wrap_idx(rpad)
            idx16[g, :, GE // 16 :] = _wrap_idx(cpad)
            rel = np.full(GE, 300.0, np.float32)
            rel[:ne] = (r - base).astype(np.float32)
            meta32[g, :, :, 0] = rel.reshape(NT, P).T  # [p, t]
            evg = np.zeros((GE, 3), np.float32)
            evg[:ne] = edge_vec_unit[eids]
            meta32[g, :, :, 1:4] = evg.reshape(NT, P, 3).transpose(1, 0, 2)
            ev16[g] = evg.reshape(NT, P, 3).transpose(1, 0, 2).astype(np.float16)
            ea_s[g * GE : g * GE + ne] = edge_attr[eids]
            last = int(r[-1])
            w = last - base + 1
            scat32[g, :w] = np.arange(base, last + 1, dtype=np.int32) - core_base

        s_res = np.zeros((NPC_PAD, DS), np.float32)
        s_res[:NPC] = s[core_base : core_base + NPC]
        deg = np.zeros((NPC_PAD, 1), np.float32)
        deg[:NPC, 0] = deg_all[core_base : core_base + NPC]
        v_cd = np.zeros((NPC_PAD, 3 * DV), np.float32)
        v_cd[:NPC] = (
            v[core_base : core_base + NPC].transpose(0, 2, 1).reshape(NPC, 3 * DV)
        )
        in_maps.append(
            dict(
                idx16=idx16,
                meta32=meta32,
                ev16=ev16,
                ea_s=ea_s,
                scat32=scat32[:, :, None],
                deg=deg,
                s_res=s_res,
                v_cd=v_cd,
            )
        )
    return G, in_maps


def _build(G):
    nc = bacc.Bacc("TRN2", target_bir_lowering=False)
    NB = 2 * G  # 512-edge blocks

    # ---- dram tensors ----
    t_s = nc.dram_tensor("s_pad", [N_PAD, DS], f32, kind="ExternalInput")
    t_idx = nc.dram_tensor("idx16", [G, P, 2 * GE // 16], i16, kind="ExternalInput")
    t_meta = nc.dram_tensor("meta32", [G, P, NT, 4], f32, kind="ExternalInput")
    t_ev = nc.dram_tensor("ev16", [G, P, NT, 3], f16, kind="ExternalInput")
    t_ea = nc.dram_tensor("ea_s", [G * GE, DS], f32, kind="ExternalInput")
    t_scat = nc.dram_tensor("scat32", [G, P, 1], i32, kind="ExternalInput")
    t_deg = nc.dram_tensor("deg", [NPC_PAD, 1], f32, kind="ExternalInput")
    t_sres = nc.dram_tensor("s_res", [NPC_PAD, DS], f32, kind="ExternalInput")
    t_vcd = nc.dram_tensor("v_cd", [NPC_PAD, 3 * DV], f32, kind="ExternalInput")
    t_w1ab = nc.dram_tensor("w1ab", [DS, 2 * DS], f16, kind="ExternalInput")
    t_w1c = nc.dram_tensor("w1c", [DS, DS], f16, kind="ExternalInput")
    t_w2 = nc.dram_tensor("w2", [DS, DS], f16, kind="ExternalInput")
    t_wv = nc.dram_tensor("wv", [DS, DV], f16, kind="ExternalInput")
    t_ws = nc.dram_tensor("ws", [DS, DS], f16, kind="ExternalInput")
    t_b1 = nc.dram_tensor("b1c", [P, 1], f32, kind="ExternalInput")
    t_b2 = nc.dram_tensor("b2c", [P, 1], f32, kind="ExternalInput")
    t_bsb = nc.dram_tensor("bs_bc", [P, DS], f32, kind="ExternalInput")
    t_b2b = nc.dram_tensor("b2_bc", [P, DS], f32, kind="ExternalInput")
    t_bvb = nc.dram_tensor("bv_bc", [P, DV], f32, kind="ExternalInput")
    t_gam = nc.dram_tensor("gam_bc", [P, DS], f32, kind="ExternalInput")
    t_bet = nc.dram_tensor("bet_bc", [P, DS], f32, kind="ExternalInput")

    dbg = "ExternalOutput" if DEBUG_DUMP else "Internal"
    t_ab = nc.dram_tensor("AB", [N_PAD, 2 * DS], f16, kind=dbg)
    t_acc = nc.dram_tensor("acc", [NPC_PAD, ACCW], f32, kind=dbg)
    t_sout = nc.dram_tensor("s_new", [NPC_PAD, DS], f32, kind="ExternalOutput")
    t_vout = nc.dram_tensor("v_new", [NPC_PAD, 3 * DV], f32, kind="ExternalOutput")

    from concourse.masks import make_identity

    with tile.TileContext(nc) as tc:
        with tc.tile_pool(name="const", bufs=1) as const:
            ident = const.tile([P, P], f16)
            make_identity(nc, ident)
            iota_i = const.tile([P, P], i32)
            nc.gpsimd.iota(iota_i, pattern=[[1, P]], base=0, channel_multiplier=0)
            iota16 = const.tile([P, P], f16)
            nc.vector.tensor_copy(iota16, iota_i)
            w1ab_sb = const.tile([DS, 2 * DS], f16)
            nc.sync.dma_start(out=w1ab_sb, in_=t_w1ab[:, :])
            w1c_sb = const.tile([DS, DS], f16)
            nc.sync.dma_start(out=w1c_sb, in_=t_w1c[:, :])
            w2_sb = const.tile([DS, DS], f16)
            nc.sync.dma_start(out=w2_sb, in_=t_w2[:, :])
            wv_sb = const.tile([DS, DV], f16)
            nc.sync.dma_start(out=wv_sb, in_=t_wv[:, :])
            ws_sb = const.tile([DS, DS], f16)
            nc.sync.dma_start(out=ws_sb, in_=t_ws[:, :])
            b1_sb = const.tile([P, 1], f32)
            nc.sync.dma_start(out=b1_sb, in_=t_b1[:, :])
            b2_sb = const.tile([P, 1], f32)
            nc.sync.dma_start(out=b2_sb, in_=t_b2[:, :])
            bsb_sb = const.tile([P, DS], f32)
            nc.sync.dma_start(out=bsb_sb, in_=t_bsb[:, :])
            b2b_sb = const.tile([P, DS], f32)
            nc.sync.dma_start(out=b2b_sb, in_=t_b2b[:, :])
            bvb_sb = const.tile([P, DV], f32)
            nc.sync.dma_start(out=bvb_sb, in_=t_bvb[:, :])
            gam_sb = const.tile([P, DS], f32)
            nc.sync.dma_start(out=gam_sb, in_=t_gam[:, :])
            bet_sb = const.tile([P, DS], f32)
            nc.sync.dma_start(out=bet_sb, in_=t_bet[:, :])
            eps_sb = const.tile([P, 1], f32)
            nc.vector.memset(eps_sb, EPS)

            # ---------------- phase 0: AB table ----------------
            with (
                tc.tile_pool(name="p0", bufs=3) as p0,
                tc.tile_pool(name="p0ps", bufs=2, space="PSUM") as p0ps,
            ):
                for i in range(N_PAD // P):
                    sl = slice(i * P, (i + 1) * P)
                    s16 = p0.tile([P, DS], f16, tag="s16")
                    nc.gpsimd.dma_start(out=s16, in_=t_s[sl, :])
                    pst = p0ps.tile([P, P], f16, tag="pst")
                    nc.tensor.matmul(pst, s16, ident, is_transpose=True,
                                     start=True, stop=True)
                    sT = p0.tile([P, P], f16, tag="sT")
                    nc.scalar.copy(sT, pst)
                    pab = p0ps.tile([P, 2 * DS], f32, tag="pab")
                    nc.tensor.matmul(pab, sT, w1ab_sb, start=True, stop=True)
                    ab16 = p0.tile([P, 2 * DS], f16, tag="ab16")
                    nc.scalar.copy(ab16, pab)
                    nc.sync.dma_start(out=t_ab[sl, :], in_=ab16)

            # ---------------- phase 1: edges ----------------
            with (
                tc.tile_pool(name="p1", bufs=3) as p1,
                tc.tile_pool(name="p1ps", bufs=1, space="PSUM") as ps1,
                tc.tile_pool(name="p1acc", bufs=2, space="PSUM") as psacc,
            ):
                for g in range(G):
                    idx_t = p1.tile([P, 2 * GE // 16], i16, tag="idx")
                    nc.sync.dma_start(out=idx_t, in_=t_idx[g, :, :])
                    meta_t = p1.tile([P, NT, 4], f32, tag="meta")
                    nc.sync.dma_start(out=meta_t, in_=t_meta[g, :, :, :])
                    ev_t = p1.tile([P, NT, 3], f16, tag="ev")
                    nc.sync.dma_start(out=ev_t, in_=t_ev[g, :, :, :])

                    ga = p1.tile([P, NT, DS], f16, tag="ga")
                    nc.gpsimd.dma_gather(
                        ga[:, :, :], t_ab[:, 0:DS], idx_t[:, 0 : GE // 16],
                        GE, GE, DS, elem_step=2 * DS,
                    )
                    gb = p1.tile([P, NT, DS], f16, tag="gb")
                    nc.gpsimd.dma_gather(
                        gb[:, :, :], t_ab[:, DS : 2 * DS],
                        idx_t[:, GE // 16 : 2 * GE // 16],
                        GE, GE, DS, elem_step=2 * DS,
                    )
                    S16 = p1.tile([P, NT * DS], f16, tag="S16")
                    nc.vector.tensor_add(
                        S16,
                        ga.rearrange("p c d -> p (c d)"),
                        gb.rearrange("p c d -> p (c d)"),
                    )

                    acc_ps = psacc.tile([P, ACCW], f32, tag="acc")

                    for blk in range(2):
                        ea16 = p1.tile([P, 4, DS], f16, tag="ea16")
                        r0 = g * GE + blk * 512
                        nc.gpsimd.dma_start(
                            out=ea16,
                            in_=t_ea[r0 : r0 + 512, :].rearrange(
                                "(c p) f -> p c f", p=P
                            ),
                        )
                        ps_eaT = ps1.tile([P, 512], f16, tag="eaT")
                        for c in range(4):
                            nc.tensor.matmul(
                                ps_eaT[:, c * P : (c + 1) * P], ea16[:, c, :],
                                ident, is_transpose=True, start=True, stop=True,
                            )
                        eaT = p1.tile([P, 512], f16, tag="eaTs")
                        nc.scalar.copy(eaT, ps_eaT)

                        ps_ST = ps1.tile([P, 512], f16, tag="ST")
                        for c in range(4):
                            t = blk * 4 + c
                            nc.tensor.matmul(
                                ps_ST[:, c * P : (c + 1) * P],
                                S16[:, t * DS : (t + 1) * DS],
                                ident, is_transpose=True, start=True, stop=True,
                            )
                        ST16 = p1.tile([P, 512], f16, tag="ST16")
                        nc.scalar.copy(ST16, ps_ST)

                        ps_u = ps1.tile([P, 512], f32, tag="u")
                        nc.tensor.matmul(ps_u, w1c_sb, eaT, start=True, stop=False)
                        nc.tensor.matmul(ps_u, ident, ST16, start=False, stop=True)
                        u16 = p1.tile([P, 512], f16, tag="u16")
                        nc.scalar.activation(
                            u16, ps_u, mybir.ActivationFunctionType.Silu,
                            bias=b1_sb, scale=1.0,
                        )
                        ps_msg = ps1.tile([P, 512], f32, tag="msg")
                        nc.tensor.matmul(ps_msg, w2_sb, u16, start=True, stop=True)
                        u2 = p1.tile([P, 512], f16, tag="u2")
                        nc.scalar.activation(
                            u2, ps_msg, mybir.ActivationFunctionType.Silu,
                            bias=b2_sb, scale=1.0,
                        )
                        ps_msgE = ps1.tile([P, 512], f32, tag="msgE")
                        for c in range(4):
                            nc.tensor.matmul(
                                ps_msgE[:, c * P : (c + 1) * P],
                                u16[:, c * P : (c + 1) * P], w2_sb,
                                start=True, stop=True,
                            )
                        ps_vg = ps1.tile([P, 4 * DV], f32, tag="vg")
                        for c in range(4):
                            nc.tensor.matmul(
                                ps_vg[:, c * DV : (c + 1) * DV],
                                u2[:, c * P : (c + 1) * P], wv_sb,
                                start=True, stop=True,
                            )
                        vg16 = p1.tile([P, 4 * DV], f16, tag="vg16")
                        nc.vector.tensor_copy(vg16, ps_vg)

                        rhs_t = p1.tile([P, 4, ACCW], f16, tag="rhs")
                        oh_t = p1.tile([P, 4, P], f16, tag="oh")
                        for c in range(4):
                            t = blk * 4 + c
                            nc.scalar.copy(
                                rhs_t[:, c, 0:DS], ps_msgE[:, c * P : (c + 1) * P]
                            )
                            for j in range(3):
                                nc.vector.tensor_scalar_mul(
                                    out=rhs_t[:, c, DS + j * DV : DS + (j + 1) * DV],
                                    in0=vg16[:, c * DV : (c + 1) * DV],
                                    scalar1=meta_t[:, t, 1 + j : 2 + j],
                                )
                            nc.vector.tensor_copy(
                                rhs_t[:, c, 320:ACCW], ev_t[:, t, :]
                            )
                            nc.vector.tensor_scalar(
                                out=oh_t[:, c, :], in0=iota16,
                                scalar1=meta_t[:, t, 0:1], scalar2=None,
                                op0=mybir.AluOpType.is_equal,
                            )
                            first = blk == 0 and c == 0
                            last = blk == 1 and c == 3
                            nc.tensor.matmul(
                                acc_ps[:, 0:ACCW], oh_t[:, c, :], rhs_t[:, c, :],
                                start=first, stop=last,
                            )

                    acc_sb = p1.tile([P, ACCW], f32, tag="accsb")
                    nc.vector.tensor_copy(acc_sb, acc_ps)
                    scat_t = p1.tile([P, 1], i32, tag="scat")
                    nc.sync.dma_start(out=scat_t, in_=t_scat[g, :, :])
                    nc.gpsimd.indirect_dma_start(
                        out=t_acc[:, :],
                        out_offset=bass.IndirectOffsetOnAxis(
                            ap=scat_t[:, 0:1], axis=0
                        ),
                        in_=acc_sb[:, :],
                        in_offset=None,
                        bounds_check=NPC_PAD - 1,
                        oob_is_err=False,
                    )

            # ---------------- phase 2: node update ----------------
            with (
                tc.tile_pool(name="p2", bufs=3) as p2,
                tc.tile_pool(name="p2ps", bufs=2, space="PSUM") as ps2,
            ):
                for i in range(NTILE_NODE):
                    sl = slice(i * P, (i + 1) * P)
                    acc_t = p2.tile([P, ACCW], f32, tag="acc")
                    nc.sync.dma_start(out=acc_t, in_=t_acc[sl, :])
                    deg_t = p2.tile([P, 1], f32, tag="deg")
                    nc.sync.dma_start(out=deg_t, in_=t_deg[sl, :])

                    # s_out = acc[:, :DS] + deg*b2
                    t0 = p2.tile([P, DS], f32, tag="t0")
                    nc.vector.tensor_scalar_mul(out=t0, in0=b2b_sb, scalar1=deg_t)
                    s_out = p2.tile([P, DS], f32, tag="sout")
                    nc.vector.tensor_add(s_out, acc_t[:, 0:DS], t0)
                    s_act = p2.tile([P, DS], f16, tag="sact")
                    nc.scalar.activation(
                        s_act, s_out, mybir.ActivationFunctionType.Silu
                    )
                    pst = ps2.tile([P, P], f16, tag="pst")
                    nc.tensor.matmul(pst, s_act, ident, is_transpose=True,
                                     start=True, stop=True)
                    sT = p2.tile([P, P], f16, tag="sT")
                    nc.scalar.copy(sT, pst)
                    ps_upd = ps2.tile([P, DS], f32, tag="upd")
                    nc.tensor.matmul(ps_upd, sT, ws_sb, start=True, stop=True)
                    s_mid = p2.tile([P, DS], f32, tag="smid")
                    nc.vector.tensor_add(s_mid, ps_upd, bsb_sb)
                    s_res = p2.tile([P, DS], f32, tag="sres")
                    nc.sync.dma_start(out=s_res, in_=t_sres[sl, :])
                    nc.vector.tensor_add(s_mid, s_mid, s_res)

                    stats = p2.tile([P, 6], f32, tag="stats")
                    nc.vector.bn_stats(out=stats, in_=s_mid)
                    mv = p2.tile([P, 2], f32, tag="mv")
                    nc.vector.bn_aggr(out=mv, in_=stats)
                    std = p2.tile([P, 1], f32, tag="std")
                    nc.scalar.activation(
                        std, mv[:, 1:2], mybir.ActivationFunctionType.Sqrt,
                        bias=eps_sb, scale=1.0,
                    )
                    rstd = p2.tile([P, 1], f32, tag="rstd")
                    nc.vector.reciprocal(rstd, std)
                    xhat = p2.tile([P, DS], f32, tag="xhat")
                    nc.vector.tensor_scalar(
                        out=xhat, in0=s_mid, scalar1=mv[:, 0:1], scalar2=rstd,
                        op0=mybir.AluOpType.subtract, op1=mybir.AluOpType.mult,
                    )
                    s_new = p2.tile([P, DS], f32, tag="snew")
                    nc.vector.tensor_mul(s_new, xhat, gam_sb)
                    nc.vector.tensor_add(s_new, s_new, bet_sb)
                    nc.sync.dma_start(out=t_sout[sl, :], in_=s_new)

                    # v
                    for j in range(3):
                        tv = p2.tile([P, DV], f32, tag="tv")
                        nc.vector.tensor_scalar_mul(
                            out=tv, in0=bvb_sb, scalar1=acc_t[:, 320 + j : 321 + j]
                        )
                        nc.vector.tensor_add(
                            acc_t[:, DS + j * DV : DS + (j + 1) * DV],
                            acc_t[:, DS + j * DV : DS + (j + 1) * DV],
                            tv,
                        )
                    v_t = p2.tile([P, 3 * DV], f32, tag="vt")
                    nc.sync.dma_start(out=v_t, in_=t_vcd[sl, :])
                    v_new = p2.tile([P, 3 * DV], f32, tag="vnew")
                    nc.vector.tensor_add(v_new, acc_t[:, DS : DS + 3 * DV], v_t)
                    nc.sync.dma_start(out=t_vout[sl, :], in_=v_new)

    nc.finalize()
    return nc


_CACHE = {}
TRACE = False
DEBUG_DUMP = False
LAST_RESULT = None


class _Runner:
    """Compiled SPMD executable with cached jit; supports timed re-execution."""

    def __init__(self, nc):
        import jax
        from jax.experimental.shard_map import shard_map
        from jax.sharding import Mesh, PartitionSpec

        from concourse import bass2jax, mybir as mb

        bass2jax.install_neuronx_cc_hook()
        part_name = (
            nc.partition_id_tensor.name if nc.partition_id_tensor else None
        )
        in_names, out_names, out_avals, zero_outs = [], [], [], []
        for alloc in nc.m.functions[0].allocations:
            if not isinstance(alloc, mb.MemoryLocationSet):
                continue
            name = alloc.memorylocations[0].name
            if alloc.kind == "ExternalInput":
                if name != part_name:
                    in_names.append(name)
            elif alloc.kind == "ExternalOutput":
                out_names.append(name)
                shape = tuple(alloc.tensor_shape)
                dtype = mb.dt.np(alloc.dtype)
                out_avals.append(jax.core.ShapedArray(shape, dtype))
                zero_outs.append(np.zeros(shape, dtype))
        self.in_names, self.out_names = in_names, out_names
        self.out_avals, self.zero_outs = out_avals, zero_outs
        n_params = len(in_names)
        all_names = in_names + out_names
        if part_name is not None:
            all_names = all_names + [part_name]
        donate = tuple(range(n_params, n_params + len(out_names)))

        def _body(*args):
            operands = list(args)
            if part_name is not None:
                operands.append(bass2jax.partition_id_tensor())
            outs = bass2jax._bass_exec_p.bind(
                *operands,
                out_avals=tuple(out_avals),
                in_names=tuple(all_names),
                out_names=tuple(out_names),
                lowering_input_output_aliases=(),
                sim_require_finite=True,
                sim_require_nnan=True,
                nc=nc,
            )
            return tuple(outs)

        devices = jax.devices()[:NCORES]
        self.mesh = Mesh(np.asarray(devices), ("core",))
        in_specs = (PartitionSpec("core"),) * (n_params + len(out_names))
        out_specs = (PartitionSpec("core"),) * len(out_names)
        self.fn = jax.jit(
            shard_map(_body, mesh=self.mesh, in_specs=in_specs,
                      out_specs=out_specs, check_rep=False),
            donate_argnums=donate,
            keep_unused=True,
        )

    def concat_inputs(self, in_maps):
        return [
            np.concatenate(
                [np.asarray(m[name]) for m in in_maps], axis=0
            )
            for name in self.in_names
        ]

    def zeros(self):
        return [
            np.zeros((NCORES * z.shape[0], *z.shape[1:]), z.dtype)
            for z in self.zero_outs
        ]

    def __call__(self, concat_in):
        out_arrs = self.fn(*concat_in, *self.zeros())
        return [
            {
                name: np.asarray(out_arrs[i]).reshape(
                    NCORES, *self.out_avals[i].shape
                )[c]
                for i, name in enumerate(self.out_names)
            }
            for c in range(NCORES)
        ]


def _get_runner(G):
    key = (G, DEBUG_DUMP)
    if key not in _CACHE:
        _CACHE[key] = _Runner(_build(G))
    return _CACHE[key]


def kernel(s, v, edge_index, edge_attr, edge_vec_unit,
           W1, b1, W2, b2, Ws, bs, Wv, bv, gamma, beta):
    s = np.asarray(s, np.float32)
    v = np.asarray(v, np.float32)
    edge_index = np.asarray(edge_index)
    edge_attr = np.asarray(edge_attr, np.float32)
    edge_vec_unit = np.asarray(edge_vec_unit, np.float32)

    G, in_maps = _host_prep(s, v, edge_index, edge_attr, edge_vec_unit)
    runner = _get_runner(G)

    s_pad = np.zeros((N_PAD, DS), np.float32)
    s_pad[:N] = s
    shared = dict(
        s_pad=s_pad,
        w1ab=np.concatenate(
            [np.asarray(W1[0:DS], np.float16), np.asarray(W1[DS : 2 * DS], np.float16)],
            axis=1,
        ),
        w1c=np.asarray(W1[2 * DS : 3 * DS], np.float16),
        w2=np.asarray(W2, np.float16),
        wv=np.asarray(Wv, np.float16),
        ws=np.asarray(Ws, np.float16),
        b1c=np.asarray(b1, np.float32).reshape(P, 1),
        b2c=np.asarray(b2, np.float32).reshape(P, 1),
        bs_bc=np.tile(np.asarray(bs, np.float32)[None, :], (P, 1)),
        b2_bc=np.tile(np.asarray(b2, np.float32)[None, :], (P, 1)),
        bv_bc=np.tile(np.asarray(bv, np.float32)[None, :], (P, 1)),
        gam_bc=np.tile(np.asarray(gamma, np.float32)[None, :], (P, 1)),
        bet_bc=np.tile(np.asarray(beta, np.float32)[None, :], (P, 1)),
    )
    full_maps = [{**shared, **m} for m in in_maps]
    concat_in = runner.concat_inputs(full_maps)
    results = runner(concat_in)

    global LAST_RESULT
    LAST_RESULT = (runner, concat_in, results)

    s_new = np.concatenate([r["s_new"][:NPC] for r in results], axis=0)
    v_cd = np.concatenate([r["v_new"][:NPC] for r in results], axis=0)
    v_new = v_cd.reshape(N, 3, DV).transpose(0, 2, 1)
    return s_new, v_new


# revision 29
# speedup vs baseline: 1.1132x; 1.1132x over previous
"""Trainium2 Bass kernel for nn_EquivariantMLP (GNN message passing).

Strategy (8 NeuronCores, edge-parallel by destination node):
- Host: sort edges by destination (row); partition nodes into 8 contiguous
  ranges with their incoming edges; pack each core's edges into groups of
  1024 whose rows span a <128-node window (nodes never split across groups);
  distribute per-core edge data (edge_attr, edge_vec, indices) sorted.
- Device phase 0: build an f16 table AB = [s@W1a | s@W1b] so the per-edge
  3x128-wide W1 matmul becomes gather+add plus one 128-wide matmul.
- Device phase 1 (per 1024-edge group): dma_gather AB[row]/AB[col] -> add;
  transpose to feature-major; += edge_attr^T @ W1c; silu -> u;
  msg = u@W2 in both feature-major (u2 = silu(msg+b2) -> vg = u2@Wv) and
  edge-major (scatter rhs); scatter [msg | vg*ev | ev] via onehot matmul
  into a PSUM window accumulator; indirect-scatter window rows to DRAM.
- Device phase 2 (per 128 nodes): s' = LN(s + silu(s_out + deg*b2)@Ws + bs);
  v'_cd = v_cd + seg(vg*ev) + bv (x) seg(ev).
"""

import sys

sys.path.insert(0, "/opt/trn_rl_repo")

import numpy as np

import concourse.bass as bass
import concourse.bacc as bacc
import concourse.tile as tile
from concourse import mybir

f16 = mybir.dt.float16
f32 = mybir.dt.float32
i16 = mybir.dt.int16
i32 = mybir.dt.int32

N, E, DS, DV = 20000, 640000, 128, 64
NCORES = 8
P = 128
GE = 1024          # edges per group (8 tiles of 128)
NT = GE // P       # tiles per group = 8
NPC = N // NCORES  # nodes per core
NPC_PAD = ((NPC + P - 1) // P) * P          # 2560
NTILE_NODE = NPC_PAD // P                   # 20
N_PAD = ((N + P - 1) // P) * P              # 20096
EPS = 1e-5
BIG = 1 << 28
ACCW = 323  # 128 msg + 192 vmsg + 3 ev


def _wrap_idx(idx_lin):
    """Wrap linear int16 indices for dma_gather: [i%16, i//16], tiled to 128."""
    n = len(idx_lin)
    a = np.asarray(idx_lin, np.int16).reshape(n // 16, 16).T  # [16, n//16]
    return np.tile(a, (8, 1))


def _host_prep(s, v, edge_index, edge_attr, edge_vec_unit):
    row = np.asarray(edge_index[0], np.int64)
    col = np.asarray(edge_index[1], np.int64)
    perm = np.argsort(row, kind="stable")
    row_s = row[perm]
    col_s = col[perm]

    core_edge_bounds = np.searchsorted(row_s, np.arange(NCORES + 1) * NPC)
    deg_all = np.bincount(row, minlength=N).astype(np.int64)
    assert (deg_all > 0).all(), "degree-0 node present; grouping patch needed"

    cores = []
    for k in range(NCORES):
        lo, hi = core_edge_bounds[k], core_edge_bounds[k + 1]
        nodes = np.arange(k * NPC, (k + 1) * NPC)
        counts = deg_all[nodes]
        groups = []  # (base_node, estart, eend) edge offsets relative to lo
        cur_base, cur_start, cur_len = None, 0, 0
        pos = 0
        for n_, c_ in zip(nodes, counts):
            c_ = int(c_)
            if cur_base is None or cur_len + c_ > GE or (n_ - cur_base) >= P:
                if cur_base is not None:
                    groups.append((cur_base, cur_start, pos))
                cur_base, cur_start, cur_len = int(n_), pos, 0
            cur_len += c_
            pos += c_
        groups.append((cur_base, cur_start, pos))
        assert pos == hi - lo
        cores.append((lo, hi, groups))

    G = max(len(c[2]) for c in cores)

    in_maps = []
    for k in range(NCORES):
        lo, hi, groups = cores[k]
        core_base = k * NPC
        idx16 = np.zeros((G, P, 2 * GE // 16), np.int16)
        meta32 = np.zeros((G, P, NT, 4), np.float32)
        meta32[:, :, :, 0] = 300.0
        ev16 = np.zeros((G, P, NT, 3), np.float16)
        ea_s = np.zeros((G * GE, DS), np.float32)
        scat32 = np.full((G, P), BIG, np.int32)
        for g, (base, es, ee) in enumerate(groups):
            ne = ee - es
            eids = perm[lo + es : lo + ee]
            r = row_s[lo + es : lo + ee]
            c = col_s[lo + es : lo + ee]
            rpad = np.zeros(GE, np.int64)
            cpad = np.zeros(GE, np.int64)
            rpad[:ne] = r
            cpad[:ne] = c
            idx16[g, :, : GE // 16] = _wrap_idx(rpad)
            idx16[g, :, GE // 16 :] = _wrap_idx(cpad)
            rel = np.full(GE, 300.0, np.float32)
            rel[:ne] = (r - base).astype(np.float32)
            meta32[g, :, :, 0] = rel.reshape(NT, P).T  # [p, t]
            evg = np.zeros((GE, 3), np.float32)
            evg[:ne] = edge_vec_unit[eids]
            meta32[g, :, :, 1:4] = evg.reshape(NT, P, 3).transpose(1, 0, 2)
            ev16[g] = evg.reshape(NT, P, 3).transpose(1, 0, 2).astype(np.float16)
            ea_s[g * GE : g * GE + ne] = edge_attr[eids]
            last = int(r[-1])
            w = last - base + 1
            scat32[g, :w] = np.arange(base, last + 1, dtype=np.int32) - core_base

        s_res = np.zeros((NPC_PAD, DS), np.float32)
        s_res[:NPC] = s[core_base : core_base + NPC]
        deg = np.zeros((NPC_PAD, 1), np.float32)
        deg[:NPC, 0] = deg_all[core_base : core_base + NPC]
        v_cd = np.zeros((NPC_PAD, 3 * DV), np.float32)
        v_cd[:NPC] = (
            v[core_base : core_base + NPC].transpose(0, 2, 1).reshape(NPC, 3 * DV)
        )
        in_maps.append(
            dict(
                idx16=idx16,
                meta32=meta32,
                ev16=ev16,
                ea_s=ea_s,
                scat32=scat32[:, :, None],
                deg=deg,
                s_res=s_res,
                v_cd=v_cd,
            )
        )
    return G, in_maps


def _build(G):
    nc = bacc.Bacc("TRN2", target_bir_lowering=False)

    t_s = nc.dram_tensor("s_pad", [N_PAD, DS], f32, kind="ExternalInput")
    t_idx = nc.dram_tensor("idx16", [G, P, 2 * GE // 16], i16, kind="ExternalInput")
    t_meta = nc.dram_tensor("meta32", [G, P, NT, 4], f32, kind="ExternalInput")
    t_ev = nc.dram_tensor("ev16", [G, P, NT, 3], f16, kind="ExternalInput")
    t_ea = nc.dram_tensor("ea_s", [G * GE, DS], f32, kind="ExternalInput")
    t_scat = nc.dram_tensor("scat32", [G, P, 1], i32, kind="ExternalInput")
    t_deg = nc.dram_tensor("deg", [NPC_PAD, 1], f32, kind="ExternalInput")
    t_sres = nc.dram_tensor("s_res", [NPC_PAD, DS], f32, kind="ExternalInput")
    t_vcd = nc.dram_tensor("v_cd", [NPC_PAD, 3 * DV], f32, kind="ExternalInput")
    t_w1ab = nc.dram_tensor("w1ab", [DS, 2 * DS], f16, kind="ExternalInput")
    t_w1c = nc.dram_tensor("w1c", [DS, DS], f16, kind="ExternalInput")
    t_w2 = nc.dram_tensor("w2", [DS, DS], f16, kind="ExternalInput")
    t_wv = nc.dram_tensor("wv", [DS, DV], f16, kind="ExternalInput")
    t_ws = nc.dram_tensor("ws", [DS, DS], f16, kind="ExternalInput")
    t_b1 = nc.dram_tensor("b1c", [P, 1], f32, kind="ExternalInput")
    t_b2 = nc.dram_tensor("b2c", [P, 1], f32, kind="ExternalInput")
    t_bsb = nc.dram_tensor("bs_bc", [P, DS], f32, kind="ExternalInput")
    t_b2b = nc.dram_tensor("b2_bc", [P, DS], f32, kind="ExternalInput")
    t_bvb = nc.dram_tensor("bv_bc", [P, DV], f32, kind="ExternalInput")
    t_gam = nc.dram_tensor("gam_bc", [P, DS], f32, kind="ExternalInput")
    t_bet = nc.dram_tensor("bet_bc", [P, DS], f32, kind="ExternalInput")

    dbg = "ExternalOutput" if DEBUG_DUMP else "Internal"
    t_ab = nc.dram_tensor("AB", [N_PAD, 2 * DS], f16, kind=dbg)
    t_acc = nc.dram_tensor("acc", [NPC_PAD, ACCW], f32, kind=dbg)
    t_sout = nc.dram_tensor("s_new", [NPC_PAD, DS], f32, kind="ExternalOutput")
    t_vout = nc.dram_tensor("v_new", [NPC_PAD, 3 * DV], f32, kind="ExternalOutput")

    from concourse.masks import make_identity

    with tile.TileContext(nc) as tc:
        with tc.tile_pool(name="const", bufs=1) as const:
            ident = const.tile([P, P], f16)
            make_identity(nc, ident)
            iota_i = const.tile([P, P], i32)
            nc.gpsimd.iota(iota_i, pattern=[[1, P]], base=0, channel_multiplier=0)
            iota16 = const.tile([P, P], f16)
            nc.vector.tensor_copy(iota16, iota_i)
            w1ab_sb = const.tile([DS, 2 * DS], f16)
            nc.sync.dma_start(out=w1ab_sb, in_=t_w1ab[:, :])
            w1c_sb = const.tile([DS, DS], f16)
            nc.sync.dma_start(out=w1c_sb, in_=t_w1c[:, :])
            w2_sb = const.tile([DS, DS], f16)
            nc.sync.dma_start(out=w2_sb, in_=t_w2[:, :])
            wv_sb = const.tile([DS, DV], f16)
            nc.sync.dma_start(out=wv_sb, in_=t_wv[:, :])
            ws_sb = const.tile([DS, DS], f16)
            nc.sync.dma_start(out=ws_sb, in_=t_ws[:, :])
            b1_sb = const.tile([P, 1], f32)
            nc.sync.dma_start(out=b1_sb, in_=t_b1[:, :])
            b2_sb = const.tile([P, 1], f32)
            nc.sync.dma_start(out=b2_sb, in_=t_b2[:, :])
            bsb_sb = const.tile([P, DS], f32)
            nc.sync.dma_start(out=bsb_sb, in_=t_bsb[:, :])
            b2b_sb = const.tile([P, DS], f32)
            nc.sync.dma_start(out=b2b_sb, in_=t_b2b[:, :])
            bvb_sb = const.tile([P, DV], f32)
            nc.sync.dma_start(out=bvb_sb, in_=t_bvb[:, :])
            gam_sb = const.tile([P, DS], f32)
            nc.sync.dma_start(out=gam_sb, in_=t_gam[:, :])
            bet_sb = const.tile([P, DS], f32)
            nc.sync.dma_start(out=bet_sb, in_=t_bet[:, :])
            eps_sb = const.tile([P, 1], f32)
            nc.vector.memset(eps_sb, EPS)

            # ---------------- phase 0: AB table ----------------
            with (
                tc.tile_pool(name="p0", bufs=3) as p0,
                tc.tile_pool(name="p0ps", bufs=2, space="PSUM") as p0ps,
            ):
                for i in range(N_PAD // P):
                    sl = slice(i * P, (i + 1) * P)
                    s16 = p0.tile([P, DS], f16, tag="s16")
                    nc.gpsimd.dma_start(out=s16, in_=t_s[sl, :])
                    pst = p0ps.tile([P, P], f16, tag="pst")
                    nc.tensor.matmul(pst, s16, ident, is_transpose=True,
                                     start=True, stop=True)
                    sT = p0.tile([P, P], f16, tag="sT")
                    nc.scalar.copy(sT, pst)
                    pab = p0ps.tile([P, 2 * DS], f32, tag="pab")
                    nc.tensor.matmul(pab, sT, w1ab_sb, start=True, stop=True)
                    ab16 = p0.tile([P, 2 * DS], f16, tag="ab16")
                    nc.scalar.copy(ab16, pab)
                    nc.sync.dma_start(out=t_ab[sl, :], in_=ab16)

            # ---------------- phase 1: edges ----------------
            with (
                tc.tile_pool(name="p1", bufs=3) as p1,
                tc.tile_pool(name="p1ps", bufs=1, space="PSUM") as ps1,
                tc.tile_pool(name="p1acc", bufs=2, space="PSUM") as psacc,
            ):
                for g in range(G):
                    idx_t = p1.tile([P, 2 * GE // 16], i16, tag="idx")
                    nc.sync.dma_start(out=idx_t, in_=t_idx[g, :, :])
                    meta_t = p1.tile([P, NT, 4], f32, tag="meta")
                    nc.sync.dma_start(out=meta_t, in_=t_meta[g, :, :, :])
                    ev_t = p1.tile([P, NT, 3], f16, tag="ev")
                    nc.sync.dma_start(out=ev_t, in_=t_ev[g, :, :, :])

                    ga = p1.tile([P, NT, DS], f16, tag="ga")
                    nc.gpsimd.dma_gather(
                        ga[:, :, :], t_ab[:, 0:DS], idx_t[:, 0 : GE // 16],
                        GE, GE, DS, elem_step=2 * DS,
                    )
                    gb = p1.tile([P, NT, DS], f16, tag="gb")
                    nc.gpsimd.dma_gather(
                        gb[:, :, :], t_ab[:, DS : 2 * DS],
                        idx_t[:, GE // 16 : 2 * GE // 16],
                        GE, GE, DS, elem_step=2 * DS,
                    )
                    S16 = p1.tile([P, NT * DS], f16, tag="S16")
                    nc.vector.tensor_add(
                        S16,
                        ga.rearrange("p c d -> p (c d)"),
                        gb.rearrange("p c d -> p (c d)"),
                    )

                    acc_ps = psacc.tile([P, ACCW], f32, tag="acc")

                    for blk in range(2):
                        ea16 = p1.tile([P, 4, DS], f16, tag="ea16")
                        r0 = g * GE + blk * 512
                        nc.gpsimd.dma_start(
                            out=ea16,
                            in_=t_ea[r0 : r0 + 512, :].rearrange(
                                "(c p) f -> p c f", p=P
                            ),
                        )
                        ps_eaT = ps1.tile([P, 512], f16, tag="eaT")
                        for c in range(4):
                            nc.tensor.matmul(
                                ps_eaT[:, c * P : (c + 1) * P], ea16[:, c, :],
                                ident, is_transpose=True, start=True, stop=True,
                            )
                        eaT = p1.tile([P, 512], f16, tag="eaTs")
                        nc.scalar.copy(eaT, ps_eaT)

                        ps_ST = ps1.tile([P, 512], f16, tag="ST")
                        for c in range(4):
                            t = blk * 4 + c
                            nc.tensor.matmul(
                                ps_ST[:, c * P : (c + 1) * P],
                                S16[:, t * DS : (t + 1) * DS],
                                ident, is_transpose=True, start=True, stop=True,
                            )
                        ST16 = p1.tile([P, 512], f16, tag="ST16")
                        nc.scalar.copy(ST16, ps_ST)

                        ps_u = ps1.tile([P, 512], f32, tag="u")
                        nc.tensor.matmul(ps_u, w1c_sb, eaT, start=True, stop=False)
                        nc.tensor.matmul(ps_u, ident, ST16, start=False, stop=True)
                        u16 = p1.tile([P, 512], f16, tag="u16")
                        nc.scalar.activation(
                            u16, ps_u, mybir.ActivationFunctionType.Silu,
                            bias=b1_sb, scale=1.0,
                        )
                        ps_msg = ps1.tile([P, 512], f32, tag="msg")
                        nc.tensor.matmul(ps_msg, w2_sb, u16, start=True, stop=True)
                        u2 = p1.tile([P, 512], f16, tag="u2")
                        nc.scalar.activation(
                            u2, ps_msg, mybir.ActivationFunctionType.Silu,
                            bias=b2_sb, scale=1.0,
                        )
                        ps_msgE = ps1.tile([P, 512], f32, tag="msgE")
                        for c in range(4):
                            nc.tensor.matmul(
                                ps_msgE[:, c * P : (c + 1) * P],
                                u16[:, c * P : (c + 1) * P], w2_sb,
                                start=True, stop=True,
                            )
                        ps_vg = ps1.tile([P, 4 * DV], f32, tag="vg")
                        for c in range(4):
                            nc.tensor.matmul(
                                ps_vg[:, c * DV : (c + 1) * DV],
                                u2[:, c * P : (c + 1) * P], wv_sb,
                                start=True, stop=True,
                            )
                        vg16 = p1.tile([P, 4 * DV], f16, tag="vg16")
                        nc.vector.tensor_copy(vg16, ps_vg)

                        rhs_t = p1.tile([P, 4, ACCW], f16, tag="rhs")
                        oh_t = p1.tile([P, 4, P], f16, tag="oh")
                        for c in range(4):
                            t = blk * 4 + c
                            nc.scalar.copy(
                                rhs_t[:, c, 0:DS], ps_msgE[:, c * P : (c + 1) * P]
                            )
                            for j in range(3):
                                nc.vector.tensor_scalar_mul(
                                    out=rhs_t[:, c, DS + j * DV : DS + (j + 1) * DV],
                                    in0=vg16[:, c * DV : (c + 1) * DV],
                                    scalar1=meta_t[:, t, 1 + j : 2 + j],
                                )
                            nc.vector.tensor_copy(
                                rhs_t[:, c, 320:ACCW], ev_t[:, t, :]
                            )
                            nc.vector.tensor_scalar(
                                out=oh_t[:, c, :], in0=iota16,
                                scalar1=meta_t[:, t, 0:1], scalar2=None,
                                op0=mybir.AluOpType.is_equal,
                            )
                            nc.tensor.matmul(
                                acc_ps[:, 0:ACCW], oh_t[:, c, :], rhs_t[:, c, :],
                                start=(blk == 0 and c == 0),
                                stop=(blk == 1 and c == 3),
                            )

                    acc_sb = p1.tile([P, ACCW], f32, tag="accsb")
                    nc.vector.tensor_copy(acc_sb, acc_ps)
                    scat_t = p1.tile([P, 1], i32, tag="scat")
                    nc.sync.dma_start(out=scat_t, in_=t_scat[g, :, :])
                    nc.gpsimd.indirect_dma_start(
                        out=t_acc[:, :],
                        out_offset=bass.IndirectOffsetOnAxis(
                            ap=scat_t[:, 0:1], axis=0
                        ),
                        in_=acc_sb[:, :],
                        in_offset=None,
                        bounds_check=NPC_PAD - 1,
                        oob_is_err=False,
                    )

            # ---------------- phase 2: node update ----------------
            with (
                tc.tile_pool(name="p2", bufs=3) as p2,
                tc.tile_pool(name="p2ps", bufs=2, space="PSUM") as ps2,
            ):
                for i in range(NTILE_NODE):
                    sl = slice(i * P, (i + 1) * P)
                    acc_t = p2.tile([P, ACCW], f32, tag="acc")
                    nc.sync.dma_start(out=acc_t, in_=t_acc[sl, :])
                    deg_t = p2.tile([P, 1], f32, tag="deg")
                    nc.sync.dma_start(out=deg_t, in_=t_deg[sl, :])

                    # s_out = acc[:, :DS] + deg*b2
                    t0 = p2.tile([P, DS], f32, tag="t0")
                    nc.vector.tensor_scalar_mul(out=t0, in0=b2b_sb, scalar1=deg_t)
                    s_out = p2.tile([P, DS], f32, tag="sout")
                    nc.vector.tensor_add(s_out, acc_t[:, 0:DS], t0)
                    s_act = p2.tile([P, DS], f16, tag="sact")
                    nc.scalar.activation(
                        s_act, s_out, mybir.ActivationFunctionType.Silu
                    )
                    pst = ps2.tile([P, P], f16, tag="pst")
                    nc.tensor.matmul(pst, s_act, ident, is_transpose=True,
                                     start=True, stop=True)
                    sT = p2.tile([P, P], f16, tag="sT")
                    nc.scalar.copy(sT, pst)
                    ps_upd = ps2.tile([P, DS], f32, tag="upd")
                    nc.tensor.matmul(ps_upd, sT, ws_sb, start=True, stop=True)
                    s_mid = p2.tile([P, DS], f32, tag="smid")
                    nc.vector.tensor_add(s_mid, ps_upd, bsb_sb)
                    s_res = p2.tile([P, DS], f32, tag="sres")
                    nc.sync.dma_start(out=s_res, in_=t_sres[sl, :])
                    nc.vector.tensor_add(s_mid, s_mid, s_res)

                    stats = p2.tile([P, 6], f32, tag="stats")
                    nc.vector.bn_stats(out=stats, in_=s_mid)
                    mv = p2.tile([P, 2], f32, tag="mv")
                    nc.vector.bn_aggr(out=mv, in_=stats)
                    std = p2.tile([P, 1], f32, tag="std")
                    nc.scalar.activation(
                        std, mv[:, 1:2], mybir.ActivationFunctionType.Sqrt,
                        bias=eps_sb, scale=1.0,
                    )
                    rstd = p2.tile([P, 1], f32, tag="rstd")
                    nc.vector.reciprocal(rstd, std)
                    xhat = p2.tile([P, DS], f32, tag="xhat")
                    nc.vector.tensor_scalar(
                        out=xhat, in0=s_mid, scalar1=mv[:, 0:1], scalar2=rstd,
                        op0=mybir.AluOpType.subtract, op1=mybir.AluOpType.mult,
                    )
                    s_new = p2.tile([P, DS], f32, tag="snew")
                    nc.vector.tensor_mul(s_new, xhat, gam_sb)
                    nc.vector.tensor_add(s_new, s_new, bet_sb)
                    nc.sync.dma_start(out=t_sout[sl, :], in_=s_new)

                    for j in range(3):
                        tv = p2.tile([P, DV], f32, tag="tv")
                        nc.vector.tensor_scalar_mul(
                            out=tv, in0=bvb_sb, scalar1=acc_t[:, 320 + j : 321 + j]
                        )
                        nc.vector.tensor_add(
                            acc_t[:, DS + j * DV : DS + (j + 1) * DV],
                            acc_t[:, DS + j * DV : DS + (j + 1) * DV],
                            tv,
                        )
                    v_t = p2.tile([P, 3 * DV], f32, tag="vt")
                    nc.sync.dma_start(out=v_t, in_=t_vcd[sl, :])
                    v_new = p2.tile([P, 3 * DV], f32, tag="vnew")
                    nc.vector.tensor_add(v_new, acc_t[:, DS : DS + 3 * DV], v_t)
                    nc.sync.dma_start(out=t_vout[sl, :], in_=v_new)

    nc.finalize()
    return nc


_CACHE = {}
TRACE = False
DEBUG_DUMP = False
LAST_RESULT = None


class _Runner:
    """Compiled SPMD executable with cached jit; supports timed re-execution."""

    def __init__(self, nc):
        import jax
        from jax.experimental.shard_map import shard_map
        from jax.sharding import Mesh, PartitionSpec

        from concourse import bass2jax, mybir as mb

        bass2jax.install_neuronx_cc_hook()
        part_name = (
            nc.partition_id_tensor.name if nc.partition_id_tensor else None
        )
        in_names, out_names, out_avals, zero_outs = [], [], [], []
        for alloc in nc.m.functions[0].allocations:
            if not isinstance(alloc, mb.MemoryLocationSet):
                continue
            name = alloc.memorylocations[0].name
            if alloc.kind == "ExternalInput":
                if name != part_name:
                    in_names.append(name)
            elif alloc.kind == "ExternalOutput":
                out_names.append(name)
                shape = tuple(alloc.tensor_shape)
                dtype = mb.dt.np(alloc.dtype)
                out_avals.append(jax.core.ShapedArray(shape, dtype))
                zero_outs.append(np.zeros(shape, dtype))
        self.in_names, self.out_names = in_names, out_names
        self.out_avals, self.zero_outs = out_avals, zero_outs
        n_params = len(in_names)
        all_names = in_names + out_names
        if part_name is not None:
            all_names = all_names + [part_name]
        donate = tuple(range(n_params, n_params + len(out_names)))

        def _body(*args):
            operands = list(args)
            if part_name is not None:
                operands.append(bass2jax.partition_id_tensor())
            outs = bass2jax._bass_exec_p.bind(
                *operands,
                out_avals=tuple(out_avals),
                in_names=tuple(all_names),
                out_names=tuple(out_names),
                lowering_input_output_aliases=(),
                sim_require_finite=True,
                sim_require_nnan=True,
                nc=nc,
            )
            return tuple(outs)

        devices = jax.devices()[:NCORES]
        self.mesh = Mesh(np.asarray(devices), ("core",))
        in_specs = (PartitionSpec("core"),) * (n_params + len(out_names))
        out_specs = (PartitionSpec("core"),) * len(out_names)
        self.fn = jax.jit(
            shard_map(_body, mesh=self.mesh, in_specs=in_specs,
                      out_specs=out_specs, check_rep=False),
            donate_argnums=donate,
            keep_unused=True,
        )

    def concat_inputs(self, in_maps):
        return [
            np.concatenate([np.asarray(m[name]) for m in in_maps], axis=0)
            for name in self.in_names
        ]

    def zeros(self):
        return [
            np.zeros((NCORES * z.shape[0], *z.shape[1:]), z.dtype)
            for z in self.zero_outs
        ]

    def __call__(self, concat_in):
        out_arrs = self.fn(*concat_in, *self.zeros())
        return [
            {
                name: np.asarray(out_arrs[i]).reshape(
                    NCORES, *self.out_avals[i].shape
                )[c]
                for i, name in enumerate(self.out_names)
            }
            for c in range(NCORES)
        ]


def _get_runner(G):
    key = (G, DEBUG_DUMP)
    if key not in _CACHE:
        _CACHE[key] = _Runner(_build(G))
    return _CACHE[key]


def kernel(s, v, edge_index, edge_attr, edge_vec_unit,
           W1, b1, W2, b2, Ws, bs, Wv, bv, gamma, beta):
    s = np.asarray(s, np.float32)
    v = np.asarray(v, np.float32)
    edge_index = np.asarray(edge_index)
    edge_attr = np.asarray(edge_attr, np.float32)
    edge_vec_unit = np.asarray(edge_vec_unit, np.float32)

    G, in_maps = _host_prep(s, v, edge_index, edge_attr, edge_vec_unit)
    runner = _get_runner(G)

    s_pad = np.zeros((N_PAD, DS), np.float32)
    s_pad[:N] = s
    shared = dict(
        s_pad=s_pad,
        w1ab=np.concatenate(
            [np.asarray(W1[0:DS], np.float16),
             np.asarray(W1[DS : 2 * DS], np.float16)],
            axis=1,
        ),
        w1c=np.asarray(W1[2 * DS : 3 * DS], np.float16),
        w2=np.asarray(W2, np.float16),
        wv=np.asarray(Wv, np.float16),
        ws=np.asarray(Ws, np.float16),
        b1c=np.asarray(b1, np.float32).reshape(P, 1),
        b2c=np.asarray(b2, np.float32).reshape(P, 1),
        bs_bc=np.tile(np.asarray(bs, np.float32)[None, :], (P, 1)),
        b2_bc=np.tile(np.asarray(b2, np.float32)[None, :], (P, 1)),
        bv_bc=np.tile(np.asarray(bv, np.float32)[None, :], (P, 1)),
        gam_bc=np.tile(np.asarray(gamma, np.float32)[None, :], (P, 1)),
        bet_bc=np.tile(np.asarray(beta, np.float32)[None, :], (P, 1)),
    )
    full_maps = [{**shared, **m} for m in in_maps]
    concat_in = runner.concat_inputs(full_maps)
    results = runner(concat_in)

    global LAST_RESULT
    LAST_RESULT = (runner, concat_in, results)

    s_new = np.concatenate([r["s_new"][:NPC] for r in results], axis=0)
    v_cd = np.concatenate([r["v_new"][:NPC] for r in results], axis=0)
    v_new = v_cd.reshape(N, 3, DV).transpose(0, 2, 1)
    return s_new, v_new
